# revision 21
# baseline (speedup 1.0000x reference)
"""CSAB (cross-set attention block) Trainium2 kernel.

Full inputs in, full outputs out. Internally: data-parallel over batch
B=8 across the 8 NeuronCores (attention and projections are
batch-independent), one batch element per core.

Per-core dataflow (all matmuls bf16, fp32 PSUM accumulation):
  - activations kept feature-major (transposed) so every matmul
    contracts over the partition dim with no on-chip transposes of the
    big score tensor:
      Q^T, K^T  [D, N]   from  lhsT=W chunks,  rhs=X^T
      V          [N, D]   token-major (lhsT=X^T chunk, rhs=W chunk),
                          augmented per-head with a ones column -> V'=[V_h|1]
      S^T[k,q]  = (K_h^T chunk).T @ Q_h^T   -- two heads of a pair run as
                  concurrent row-tiled matmuls (partitions 0:64 / 64:128)
      E^T       = exp(S^T / sqrt(D))        -- ScalarE, scale folded into
                  the activation's affine stage, no max-subtraction needed
                  (|S|/sqrt(D) < ~1)
      o'^T[65,q] = V'_h.T @ E_h^T           -- row 64 = softmax denominator
      normalize (feature-major, q-half outer so all 8 heads batch):
        o' rows are copied out of PSUM immediately (frees the bank for
        the next AV accumulation); the 8 denominator rows are staged
        into one partition's free dim [1,8,512] (engine writes must be
        32-aligned in partitions), DMA-repacked to [8,512], one batched
        DVE reciprocal (free-size bound: one [8,512] recip costs the
        same as one [1,512]), bounced through DRAM for the zero-stride
        partition broadcast, then per head:
        attn = (o' * rbc + bv) + q  -- DVE mult + fused
               scalar_tensor_tensor (V-bias folded here: softmax weights
               sum to 1, so sum A (v + bv) = sum A v + bv)
      fc: out^T accumulates W.T @ attn^T plus W.T @ Q^T (the attention
      q-residual passes through fc linearly), then bias+relu+X residual.
"""

import os
import sys
import math

import numpy as np
import ml_dtypes

import concourse.bass as bass
import concourse.mybir as mybir
import concourse.tile as tile
from concourse.bass_utils import run_bass_kernel_spmd

B, N, D, H = 8, 1024, 512, 8
DH = D // H          # 64
P = 128
KC = D // P          # 4 feature chunks
QH = N // 512        # 2 q halves
KT = N // P          # 8 k tiles
NPAIR = H // 2       # 4 head pairs
SCALE = 1.0 / math.sqrt(D)

F32 = mybir.dt.float32
BF16 = mybir.dt.bfloat16
FP8 = mybir.dt.float8e4
AF = mybir.ActivationFunctionType
ALU = mybir.AluOpType

_BRANCHES = [("xx", "x", "x"), ("xy", "x", "y"), ("yx", "y", "x"), ("yy", "y", "y")]

LAST_RESULT = None
_CACHED_NC = None


def _split_excess_waits(nc):
    """The walrus build in this container accepts at most one sync-wait
    per instruction (two for EventSemaphore). Tile's scheduler emits
    several on some instructions. Hoist the overflow onto same-engine
    NoOps inserted immediately before the instruction — the engine
    blocks at the nops instead, so the wait point in the instruction
    stream is unchanged."""
    cap_of = {"InstEventSemaphore": 2}

    def cap_for(inst):
        if getattr(inst, "is_scalar_tensor_tensor", False):
            return 0   # the STT ISA struct has no sync-wait slot
        return cap_of.get(type(inst).__name__, 1)
    # Pass 1: strip overflow waits off each instruction, remember them.
    plans = []
    for f in nc.m.functions:
        for bb in f.blocks:
            plan = []
            for inst in list(bb.instructions):
                si = getattr(inst, "sync_info", None)
                waits = list(si.on_wait) if si and si.on_wait else []
                cap = cap_for(inst)
                if len(waits) > cap:
                    cut = len(waits) - cap
                    plan.append((inst, waits[:cut]))
                    si.on_wait = waits[cut:]
            plans.append((bb, plan))
    # Pass 2: create the carrier nops. The engine builder appends them to
    # whatever block is current — they are stripped by name in pass 3 and
    # re-inserted at their proper position.
    nop_map = {}
    created = set()
    for bb, plan in plans:
        for inst, extra in plan:
            nops = []
            for w in extra:
                ni = nc.engines[inst.engine].nop(hint="waitsplit")
                ni.ins.sync_info = mybir.SyncInfo(on_wait=[w], on_update=[])
                nops.append(ni.ins)
                created.add(ni.ins.name)
            nop_map[inst.name] = nops
    # Pass 3: rebuild each block: drop stray auto-appended copies, insert
    # each nop chain immediately before its instruction.
    for bb, plan in plans:
        live = [i for i in bb.instructions if i.name not in created]
        new = []
        for inst in live:
            new.extend(nop_map.get(inst.name, ()))
            new.append(inst)
        bb.instructions = new


def _build_nc():
    nc = bass.Bass()

    # ---- DRAM I/O (per core) -------------------------------------------
    xt_bf = nc.dram_tensor("xt_bf", [D, N], BF16, kind="ExternalInput")
    yt_bf = nc.dram_tensor("yt_bf", [D, N], BF16, kind="ExternalInput")
    xt_f32 = nc.dram_tensor("xt_f32", [D, N], F32, kind="ExternalInput")
    yt_f32 = nc.dram_tensor("yt_f32", [D, N], F32, kind="ExternalInput")
    wdr = {}
    for bn, _, _ in _BRANCHES:
        for t in ("q", "k", "v"):
            wdr[f"w{t}_{bn}"] = nc.dram_tensor(f"w{t}_{bn}", [D, D], BF16,
                                               kind="ExternalInput")
            wdr[f"b{t}_{bn}"] = nc.dram_tensor(f"b{t}_{bn}", [D], F32,
                                               kind="ExternalInput")
    wfc_x = nc.dram_tensor("wfc_x", [2 * D, D], BF16, kind="ExternalInput")
    wfc_y = nc.dram_tensor("wfc_y", [2 * D, D], BF16, kind="ExternalInput")
    bfc_x = nc.dram_tensor("bfc_x", [D], F32, kind="ExternalInput")
    bfc_y = nc.dram_tensor("bfc_y", [D], F32, kind="ExternalInput")
    out_x_t = nc.dram_tensor("out_x_t", [D, N], F32, kind="ExternalOutput")
    out_y_t = nc.dram_tensor("out_y_t", [D, N], F32, kind="ExternalOutput")

    with tile.TileContext(nc) as tc, \
         tc.tile_pool(name="const", bufs=1) as const_pool, \
         tc.tile_pool(name="acts", bufs=1) as acts_pool, \
         tc.tile_pool(name="wbr", bufs=2) as wbr_pool, \
         tc.tile_pool(name="qkv_q", bufs=2) as q_pool, \
         tc.tile_pool(name="qkv_kv", bufs=2) as kv_pool, \
         tc.tile_pool(name="epool", bufs=4) as e_pool, \
         tc.tile_pool(name="norm", bufs=4) as norm_pool, \
         tc.tile_pool(name="norm2", bufs=2) as norm2_pool, \
         tc.tile_pool(name="dstage", bufs=2) as dstage_pool, \
         tc.tile_pool(name="attn", bufs=3) as attn_pool, \
         tc.tile_pool(name="stream", bufs=2) as stream_pool, \
         tc.tile_pool(name="dbounce", bufs=2, space="DRAM") as dram_pool, \
         tc.tile_pool(name="qk_ps", bufs=2, space="PSUM") as qk_ps_pool, \
         tc.tile_pool(name="misc_ps", bufs=2, space="PSUM") as misc_ps_pool, \
         tc.tile_pool(name="proj_ps", bufs=2, space="PSUM") as proj_ps_pool:

        # ---- resident activations --------------------------------------
        xt_sb = acts_pool.tile([P, KC, N], BF16, tag="xt_sb")
        yt_sb = acts_pool.tile([P, KC, N], BF16, tag="yt_sb")
        for kc in range(KC):
            # chunked loads so the first projection matmuls start early
            nc.sync.dma_start(
                xt_sb[:, kc, :],
                xt_bf.rearrange("(o p) n -> p o n", p=P)[:, kc, :])
            nc.sync.dma_start(
                yt_sb[:, kc, :],
                yt_bf.rearrange("(o p) n -> p o n", p=P)[:, kc, :])
        act_sb = {"x": xt_sb, "y": yt_sb}
        act_res_dram = {"x": xt_f32, "y": yt_f32}

        wfc_sb = {}
        bfc_sb = {}
        for nm, wd, bd in (("x", wfc_x, bfc_x), ("y", wfc_y, bfc_y)):
            w = const_pool.tile([P, 2 * KC, D], BF16, tag=f"wfc_{nm}")
            nc.sync.dma_start(w[:], wd.rearrange("(o p) f -> p o f", p=P))
            bt = const_pool.tile([P, KC], F32, tag=f"bfc_{nm}")
            nc.sync.dma_start(bt[:], bd.rearrange("(o p) -> p o", p=P))
            wfc_sb[nm] = w
            bfc_sb[nm] = bt

        def proj_closures(st, qsrc, kvsrc):
            # Build this branch's projection work as a list of small PE
            # groups so they can be interleaved into the previous
            # branch's exp-bound pair loop (the PE queue is in-order).
            qt_sb = q_pool.tile([P, KC, N], BF16, tag="qt")
            kt_sb = kv_pool.tile([P, KC, N], BF16, tag="kt")
            vaug_sb = kv_pool.tile([P, KT, H * (DH + 1)], BF16, tag="vaug")
            st.update(qt=qt_sb, kt=kt_sb, vaug=vaug_sb)
            gs = []

            def fm_group(src_sb, w_sb, b_sb, out_sb, ofc, qh):
                ps = proj_ps_pool.tile([P, 512], F32, tag="proj")
                for kc in range(KC):
                    nc.tensor.matmul(
                        ps[:],
                        lhsT=w_sb[:, kc, ofc * P:(ofc + 1) * P],
                        rhs=src_sb[:, kc, qh * 512:(qh + 1) * 512],
                        start=(kc == 0), stop=(kc == KC - 1),
                    )
                nc.vector.tensor_scalar_add(
                    out_sb[:, ofc, qh * 512:(qh + 1) * 512],
                    ps[:], b_sb[:, ofc:ofc + 1],
                )

            def v_group(src_sb, w_sb, bvb_sb, tt, last):
                ps = proj_ps_pool.tile([P, 512], F32, tag="proj")
                for kc in range(KC):
                    nc.tensor.matmul(
                        ps[:],
                        lhsT=src_sb[:, kc, tt * P:(tt + 1) * P],
                        rhs=w_sb[:, kc, :],
                        start=(kc == 0), stop=(kc == KC - 1),
                    )
                dst = vaug_sb[:, tt, :].rearrange("p (h c) -> p h c", c=DH + 1)
                nc.vector.tensor_tensor(
                    dst[:, :, :DH],
                    ps[:].rearrange("p (h c) -> p h c", c=DH),
                    bvb_sb[:].rearrange("p (h c) -> p h c", c=DH),
                    ALU.add,
                )
                if last:
                    nc.gpsimd.memset(
                        vaug_sb[:].rearrange(
                            "p t (h c) -> p t h c", c=DH + 1)[:, :, :, DH:],
                        1.0,
                    )

            for w, b, out, srcn in ((st["w"]["q"], st["bq"], qt_sb, qsrc),
                                    (st["w"]["k"], st["bk"], kt_sb, kvsrc)):
                for ofc in range(KC):
                    for qh in range(QH):
                        gs.append(lambda w=w, b=b, out=out, srcn=srcn,
                                  ofc=ofc, qh=qh:
                                  fm_group(act_sb[srcn], w, b, out, ofc, qh))
            for tt in range(KT):
                gs.append(lambda tt=tt, kvsrc=kvsrc:
                          v_group(act_sb[kvsrc], st["w"]["v"], st["bvb"],
                                  tt, tt == KT - 1))
            return gs

        attn_sb_of = {}   # branch name -> O^T (= attn + Q residual) sbuf tile

        def fc_closures(out_name, br0, br1, qhg):
            return [lambda ofc=ofc: fc_one(out_name, br0, br1, qhg, ofc)
                    for ofc in range(KC)]

        def fc_one(out_name, br0, br1, qhg, ofc_only):
            # out^T[of,q] = relu(sum W.T @ O^T + b) + act^T ; DMA out
            w_sb = wfc_sb[out_name]
            b_sb = bfc_sb[out_name]
            out_dram = out_x_t if out_name == "x" else out_y_t
            res_dram = act_res_dram[out_name]
            srcs = [(attn_sb_of[br0], 0), (attn_sb_of[br1], KC)]
            for ofc in (ofc_only,):
                for qh in (qhg,):
                    # fc reuses the (idle at this point) qk pool's banks:
                    # only the first 512 columns of the [P,1024] tile.
                    ps_t = qk_ps_pool.tile([P, 1024], F32, tag="qk")
                    ps = ps_t[:, 0:512]
                    step = 0
                    for src_sb, wbase in srcs:
                        for kc in range(KC):
                            nc.tensor.matmul(
                                ps,
                                lhsT=w_sb[:, wbase + kc, ofc * P:(ofc + 1) * P],
                                rhs=src_sb[:, kc, qh * 512:(qh + 1) * 512],
                                start=(step == 0), stop=(step == 2 * KC - 1),
                            )
                            step += 1
                    xres = stream_pool.tile([P, 512], F32, tag="xres")
                    nc.sync.dma_start(
                        xres[:],
                        res_dram[ofc * P:(ofc + 1) * P, qh * 512:(qh + 1) * 512],
                    )
                    outt = stream_pool.tile([P, 512], F32, tag="outt")
                    nc.vector.tensor_scalar(
                        outt[:], ps,
                        b_sb[:, ofc:ofc + 1], 0.0,
                        ALU.add, ALU.max,
                    )
                    nc.vector.tensor_tensor(outt[:], outt[:], xres[:], ALU.add)
                    nc.sync.dma_start(
                        out_dram[ofc * P:(ofc + 1) * P, qh * 512:(qh + 1) * 512],
                        outt[:],
                    )

        # ---- branches ---------------------------------------------------
        # The PE queue is in-order, and the pair loop is ScalarE-exp
        # bound (~8.9us exp vs ~5.2us of matmul per pair).  Interleave
        # the NEXT branch's projection groups and any ready fc groups
        # between pairs so the PE never idles waiting on exp.
        fillq = []
        fcq = []
        states = {}

        def emit_wdma(bname):
            w_sb = {}
            for t in ("q", "k", "v"):
                w = wbr_pool.tile([P, KC, D], BF16, tag=f"w{t}")
                nc.sync.dma_start(
                    w[:], wdr[f"w{t}_{bname}"].rearrange("(o p) f -> p o f", p=P))
                w_sb[t] = w
            bq_sb = wbr_pool.tile([P, KC], F32, tag="bq")
            nc.sync.dma_start(
                bq_sb[:], wdr[f"bq_{bname}"].rearrange("(o p) -> p o", p=P))
            bk_sb = wbr_pool.tile([P, KC], F32, tag="bk")
            nc.sync.dma_start(
                bk_sb[:], wdr[f"bk_{bname}"].rearrange("(o p) -> p o", p=P))
            bvb_sb = wbr_pool.tile([P, D], BF16, tag="bvb")
            nc.gpsimd.dma_start(
                out=bvb_sb[:],
                in_=wdr[f"bv_{bname}"][:].partition_broadcast(P),
            )
            return dict(w=w_sb, bq=bq_sb, bk=bk_sb, bvb=bvb_sb)

        def take_fill(n):
            for _ in range(n):
                if fillq:
                    fillq.pop(0)()

        for bi, (bname, qsrc, kvsrc) in enumerate(_BRANCHES):
            if bi == 0:
                states[bname] = emit_wdma(bname)
                for g in proj_closures(states[bname], qsrc, kvsrc):
                    g()
            st = states[bname]
            if bi + 1 < len(_BRANCHES):
                nb, nq, nkv = _BRANCHES[bi + 1]
                states[nb] = emit_wdma(nb)
                fillq.extend(proj_closures(states[nb], nq, nkv))

            qt_sb, kt_sb, vaug_sb = st["qt"], st["kt"], st["vaug"]
            attn_sb = attn_pool.tile([P, KC, N], BF16, tag="attn")
            attn_sb_of[bname] = attn_sb

            for qh in range(QH):
                if qh == 1 and bname == "yy":
                    fcq.extend(fc_closures("y", "yx", "yy", 0))
                qcols = slice(qh * 512, (qh + 1) * 512)
                dstage = dstage_pool.tile([1, H, 512], BF16, tag="dstage")
                obufs = []
                for pair in range(NPAIR):
                    e_sb = e_pool.tile([P, KT, 2, 512], FP8, tag="e")
                    for kt in range(KT):
                        ps = qk_ps_pool.tile([P, 1024], F32, tag="qk")
                        nc.tensor.matmul(
                            ps[:, 0:512],
                            lhsT=kt_sb[0:DH, pair, kt * P:(kt + 1) * P],
                            rhs=qt_sb[0:DH, pair, qcols],
                            start=True, stop=True,
                        )
                        nc.tensor.matmul(
                            ps[:, 512:1024],
                            lhsT=kt_sb[DH:P, pair, kt * P:(kt + 1) * P],
                            rhs=qt_sb[DH:P, pair, qcols],
                            start=True, stop=True,
                        )
                        nc.scalar.activation(
                            e_sb[:, kt, :, :], ps[:],
                            AF.Exp, scale=SCALE,
                        )
                    obuf = norm_pool.tile([P, 512], BF16, tag="obuf")
                    obufs.append(obuf)
                    for hl in range(2):
                        h = 2 * pair + hl
                        rows = slice(hl * DH, (hl + 1) * DH)
                        ps_av = misc_ps_pool.tile([P, 512], F32, tag="misc")
                        for kt in range(KT):
                            nc.tensor.matmul(
                                ps_av[:DH + 1, :],
                                lhsT=vaug_sb[:, kt, h * (DH + 1):(h + 1) * (DH + 1)],
                                rhs=e_sb[:, kt, hl, :],
                                start=(kt == 0), stop=(kt == KT - 1),
                            )
                        nc.vector.tensor_copy(obuf[rows, :], ps_av[:DH, :])
                        nc.vector.tensor_copy(
                            dstage[0:1, h, :], ps_av[DH:DH + 1, :])
                    take_fill(3)
                    if pair % 2 == 1 and fcq:
                        fcq.pop(0)()
                # batched softmax normalize for the whole q-half
                d_d = dram_pool.tile([H, 512], BF16, tag="dd")
                nc.sync.dma_start(d_d[:], dstage[0:1, :, :])
                dp8 = norm2_pool.tile([H, 512], BF16, tag="dp8")
                nc.sync.dma_start(dp8[:, :], d_d[:])
                rp8 = norm2_pool.tile([H, 512], F32, tag="rp8")
                nc.vector.reciprocal(rp8[:], dp8[:])
                rb_d = dram_pool.tile([H, 512], F32, tag="rbd")
                nc.sync.dma_start(rb_d[:], rp8[:])
                for pair in range(NPAIR):
                    rbc = norm_pool.tile([P, 512], F32, tag="rbc")
                    for hl in range(2):
                        h = 2 * pair + hl
                        rows = slice(hl * DH, (hl + 1) * DH)
                        nc.gpsimd.dma_start(
                            out=rbc[rows, :],
                            in_=rb_d[h, :].partition_broadcast(DH))
                    nc.gpsimd.tensor_tensor(
                        attn_sb[:, pair, qcols],
                        obufs[pair][:, :], rbc[:, :], ALU.mult)
                    nc.gpsimd.tensor_tensor(
                        attn_sb[:, pair, qcols],
                        attn_sb[:, pair, qcols],
                        qt_sb[:, pair, qcols], ALU.add)

            if bname == "xy":
                fcq.extend(fc_closures("x", "xx", "xy", 0))
                fcq.extend(fc_closures("x", "xx", "xy", 1))

        for g in fillq:
            g()
        for g in fcq:
            g()
        for g in fc_closures("y", "yx", "yy", 1):
            g()

    _split_excess_waits(nc)
    return nc


def _get_nc():
    global _CACHED_NC
    if _CACHED_NC is None:
        _CACHED_NC = _build_nc()
    return _CACHED_NC


def kernel(**inputs):
    global LAST_RESULT
    nc = _get_nc()

    X = np.asarray(inputs["X"], np.float32)
    Y = np.asarray(inputs["Y"], np.float32)

    def bf(a):
        return np.ascontiguousarray(a).astype(ml_dtypes.bfloat16)

    shared = {}
    for bn, _, _ in _BRANCHES:
        for t in ("q", "k", "v"):
            shared[f"w{t}_{bn}"] = bf(inputs[f"W_{t}_{bn}"])
            shared[f"b{t}_{bn}"] = np.asarray(inputs[f"b_{t}_{bn}"], np.float32)
    shared["wfc_x"] = bf(inputs["W_X"])
    shared["wfc_y"] = bf(inputs["W_Y"])
    shared["bfc_x"] = np.asarray(inputs["b_X"], np.float32)
    shared["bfc_y"] = np.asarray(inputs["b_Y"], np.float32)

    in_maps = []
    for b in range(B):
        xt = np.ascontiguousarray(X[b].T)
        yt = np.ascontiguousarray(Y[b].T)
        m = dict(shared)
        m["xt_bf"] = xt.astype(ml_dtypes.bfloat16)
        m["yt_bf"] = yt.astype(ml_dtypes.bfloat16)
        m["xt_f32"] = xt
        m["yt_f32"] = yt
        in_maps.append(m)

    res = run_bass_kernel_spmd(nc, in_maps, list(range(B)))
    LAST_RESULT = res

    out_x = np.stack([res.results[b]["out_x_t"].T for b in range(B)])
    out_y = np.stack([res.results[b]["out_y_t"].T for b in range(B)])
    return out_x.astype(np.float32), out_y.astype(np.float32)
